# revision 13
# baseline (speedup 1.0000x reference)
"""GQA attention (tanh-score + static bias, no softmax) on 8 trn2 cores.

Reference shapes: x [4,32,256,512], H=8 heads, G=2 kv groups, D=64, N=256.
Strategy: data-parallel over the 128 (b,t) pairs -> 16 per core, zero
collectives.  All SBUF operands fp16 (1 cycle/row matmuls at any free size,
FWL weight loads, half the DMA of fp32), PSUM accumulation fp32.

Per (b,t):
  q^T = Wq^T x^T, k^T = Wk^T x^T   (feature-major, host-transposed x)
  v   = x @ Wv                      (token-major, lhsT = x^T chunks)
  sv^T = (sgr @ v)^T                (lhsT = v, rhs = sgr^T; both groups at once)
  scores^T[m,n] = k_g q_h^T         (K=64 contraction -> ROW-TILED: heads h and
                                     h+4 run concurrently in array halves)
  attn^T = tanh(scores^T * 0.125)   (ACT engine, scale fused, [128,1024]
                                     two-bank tiles to amortize overhead)
  out_h^T = v_g^T attn_h^T + sv^T   (COL-TILED: heads h and h+4 in col halves,
                                     zero wasted array; sv add fused into the
                                     PSUM->SBUF evacuation on DVE)
  y = out @ Wo                      (token-major y, Wo host-permuted)

Host-side prep: x fp16 feature-major pre-tiled; Wq columns permuted to head
pairs (h, h+4) so each q tile holds one head per array half matching the k
group layout (no doubled Wk needed); Wo rows same permutation; sgr transposed.
"""

import os
import sys

import numpy as np

for _p in ("/opt/trn_rl_repo",):
    if _p not in sys.path and os.path.isdir(_p):
        sys.path.insert(0, _p)

import concourse.bass as bass
import concourse.tile as tile
from concourse import bacc, mybir
from concourse.bass_utils import run_bass_kernel_spmd

F32 = mybir.dt.float32
F16 = mybir.dt.float16

B, T, N, C = 4, 32, 256, 512
H, G, D = 8, 2, 64
NCORES = 8
BT = B * T                      # 128
PER_CORE = BT // NCORES         # 16
NPAIR = PER_CORE // 2           # 8 iterations of 2 (b,t) each
SCALE = D ** -0.5               # 0.125

_cached = {}


def _build_nc():
    """Build + lower the single-core SPMD program."""
    nc = bacc.Bacc("TRN2", target_bir_lowering=False, debug=False,
                   num_devices=NCORES)

    # DRAM I/O (per-core shard, host-side pre-arranged, fp16)
    # xT[i, p, c, 256*b + n] = x[bt=2i+b, tok=n, cin=128c+p]
    xT = nc.dram_tensor("xT", [NPAIR, 128, 4, 512], F16, kind="ExternalInput").ap()
    sgrT = nc.dram_tensor("sgrT", [N, N], F16, kind="ExternalInput").ap()
    Wqp = nc.dram_tensor("Wqp", [C, C], F16, kind="ExternalInput").ap()
    Wk = nc.dram_tensor("Wk", [C, G * D], F16, kind="ExternalInput").ap()
    Wv = nc.dram_tensor("Wv", [C, G * D], F16, kind="ExternalInput").ap()
    Wop = nc.dram_tensor("Wop", [C, C], F16, kind="ExternalInput").ap()
    y = nc.dram_tensor("y", [PER_CORE, N, C], F16, kind="ExternalOutput").ap()

    with tile.TileContext(nc) as tc:
        _body(tc, xT, sgrT, Wqp, Wk, Wv, Wop, y)

    nc.compile()
    return nc


def _body(tc, xT, sgrT, Wqp, Wk, Wv, Wop, y):
    nc = tc.nc
    mm = nc.tensor.matmul
    import contextlib
    ctx = contextlib.ExitStack()
    with ctx:
        consts = ctx.enter_context(tc.tile_pool(name="consts", bufs=1))
        xpool = ctx.enter_context(tc.tile_pool(name="xt", bufs=3))
        qpool = ctx.enter_context(tc.tile_pool(name="qs", bufs=8))
        kpool = ctx.enter_context(tc.tile_pool(name="ks", bufs=2))
        vpool = ctx.enter_context(tc.tile_pool(name="vs", bufs=2))
        svpool = ctx.enter_context(tc.tile_pool(name="svs", bufs=2))
        apool = ctx.enter_context(tc.tile_pool(name="attn", bufs=5))
        ppool = ctx.enter_context(tc.tile_pool(name="pairs", bufs=6))
        ypool = ctx.enter_context(tc.tile_pool(name="ys", bufs=4))
        # PSUM: 8 banks of [128, 512] fp32.
        #   psA 2 banks (q / k / y cycling), psV 1 bank (v then sv),
        #   psS 2x two-bank tiles (scores), psP 1 bank = 2 attn@v tiles.
        psA = ctx.enter_context(
            tc.tile_pool(name="psA", bufs=2, space=bass.MemorySpace.PSUM))
        psV = ctx.enter_context(
            tc.tile_pool(name="psV", bufs=1, space=bass.MemorySpace.PSUM))
        psS = ctx.enter_context(
            tc.tile_pool(name="psS", bufs=2, space=bass.MemorySpace.PSUM))
        psP = ctx.enter_context(
            tc.tile_pool(name="psP", bufs=1, space=bass.MemorySpace.PSUM))

        # ---- per-iteration x prefetch (issued ahead of the consts so the
        # first q matmuls have both operands as early as possible) ----
        xts = [None] * NPAIR

        def fetch_x(it):
            t = xpool.tile([128, 4, 512], F16, tag="xt")
            nc.sync.dma_start(t[:], xT[it])
            xts[it] = t

        fetch_x(0)

        # ---- resident constants (issued on the ACT hwdge queue so their
        # descriptor setup overlaps the x DMA on the sync queue) ----
        wq = []
        wk = []
        wv = []
        wo = []
        for c in range(4):
            t = consts.tile([128, 512], F16, tag=f"wq{c}")
            nc.sync.dma_start(t[:], Wqp[128 * c:128 * (c + 1), :])
            wq.append(t)
        for c in range(4):
            t = consts.tile([128, 128], F16, tag=f"wk{c}")
            nc.sync.dma_start(t[:], Wk[128 * c:128 * (c + 1), :])
            wk.append(t)
            t = consts.tile([128, 128], F16, tag=f"wv{c}")
            nc.sync.dma_start(t[:], Wv[128 * c:128 * (c + 1), :])
            wv.append(t)
        sgt = []
        for mc in range(2):
            t = consts.tile([128, 256], F16, tag=f"sgt{mc}")
            nc.sync.dma_start(t[:], sgrT[128 * mc:128 * (mc + 1), :])
            sgt.append(t)
        for c in range(4):
            t = consts.tile([128, 512], F16, tag=f"wo{c}")
            nc.sync.dma_start(t[:], Wop[128 * c:128 * (c + 1), :])
            wo.append(t)
        fetch_x(1)

        # ---- HAM warm-up: dense dummy matmuls while the first DMAs are in
        # flight, so the PE clock gate is already at 8/8 when real work
        # starts (the activity window needs ~3.4us of sustained busy).
        dummy = consts.tile([128, 512], F16, tag="dummy")
        nc.gpsimd.memset(dummy[:], 0.0)
        wps = psP.tile([128, 512], F32, tag="psP")
        for _ in range(10):
            mm(wps[:], dummy[:, 0:128], dummy[:], start=True, stop=True)

        # per-iteration state handed between pipeline stages
        state = [None] * NPAIR

        # ---------- stage A pieces (projections for iteration it) ----------
        def proj_q(it, j):
            xt = xts[it]
            ps = psA.tile([128, 512], F32, tag="psA")
            for c in range(4):
                mm(ps[:], wq[c][:, 128 * j:128 * (j + 1)], xt[:, c, :],
                   start=(c == 0), stop=(c == 3))
            s = qpool.tile([128, 512], F16, tag="qs")
            nc.scalar.copy(s[:], ps[:])
            return s

        def proj_k(it):
            xt = xts[it]
            ps = psA.tile([128, 512], F32, tag="psA")
            for c in range(4):
                mm(ps[:], wk[c][:], xt[:, c, :],
                   start=(c == 0), stop=(c == 3))
            s = kpool.tile([128, 512], F16, tag="ks")
            nc.vector.tensor_copy(s[:], ps[:])
            return s

        def proj_v(it):
            # token-major v: block 2b+mc at cols 128*(2b+mc) holds
            # v[tok chunk mc of bt b, (g0 d | g1 d)]
            xt = xts[it]
            ps = psV.tile([128, 512], F32, tag="psV")
            for blk in range(4):
                b, mcc = blk // 2, blk % 2
                off = 256 * b + 128 * mcc
                for c in range(4):
                    mm(ps[:, 128 * blk:128 * (blk + 1)],
                       xt[:, c, off:off + 128], wv[c][:],
                       start=(c == 0), stop=(c == 3))
            s = vpool.tile([128, 512], F16, tag="vs")
            nc.vector.tensor_copy(s[:], ps[:])
            return s

        def proj_sv(it, v_sb):
            # sv^T[dpair, n] for both bt: (sgr@v_g)^T rows 64g:64g+64
            ps = psV.tile([128, 512], F32, tag="psV")
            for b in range(2):
                for mc in range(2):
                    mm(ps[:, 256 * b:256 * (b + 1)],
                       v_sb[:, 128 * (2 * b + mc):128 * (2 * b + mc + 1)],
                       sgt[mc][:], start=(mc == 0), stop=(mc == 1))
            s = svpool.tile([128, 512], F32, tag="svs")
            nc.vector.tensor_copy(s[:], ps[:])
            return s

        # ---------- stage B pieces (attention for iteration it, bt b) ------
        def scores_j(qs, ks, b, j):
            # one two-bank psum tile: cols [mc*256] head j (rows = m chunk),
            # cols [512 + mc*256] head j+4.  Heads j / j+4 run in array row
            # halves 0:64 / 64:128 concurrently (row tiling, K=64).
            ps = psS.tile([128, 1024], F32, tag="psS")
            for mc in range(2):
                off = 256 * b + 128 * mc
                mm(ps[:, 256 * mc:256 * (mc + 1)],
                   ks[0:64, off:off + 128],
                   qs[j][0:64, 256 * b:256 * (b + 1)], start=True, stop=True)
                mm(ps[:, 512 + 256 * mc:512 + 256 * (mc + 1)],
                   ks[64:128, off:off + 128],
                   qs[j][64:128, 256 * b:256 * (b + 1)], start=True, stop=True)
            a = apool.tile([128, 1024], F16, tag="attn")
            nc.scalar.activation(a[:], ps[:],
                                 mybir.ActivationFunctionType.Tanh,
                                 scale=SCALE)
            return a

        def attnv_pair(v_sb, sv_sb, b, aj, ajp):
            # col-tiled: head j -> psum rows 0:64 (array cols 0:64), head j+4
            # -> rows 64:128.  lhsT = the head's own 64 v columns, so nothing
            # is wasted; accumulate over the two m chunks.  One full psum bank
            # holds two j's (cols 0:256 / 256:512).
            ps = psP.tile([128, 512], F32, tag="psP")
            for half, a in ((0, aj), (256, ajp)):
                for mc in range(2):
                    vblk = v_sb[:, 128 * (2 * b + mc):128 * (2 * b + mc + 1)]
                    mm(ps[0:64, half:half + 256], vblk[:, 0:64],
                       a[:, 256 * mc:256 * (mc + 1)],
                       start=(mc == 0), stop=(mc == 1))
                for mc in range(2):
                    vblk = v_sb[:, 128 * (2 * b + mc):128 * (2 * b + mc + 1)]
                    mm(ps[64:128, half:half + 256], vblk[:, 64:128],
                       a[:, 512 + 256 * mc:512 + 256 * (mc + 1)],
                       start=(mc == 0), stop=(mc == 1))
            out = []
            for half in (0, 256):
                s = ppool.tile([128, 256], F16, tag="pairs")
                nc.vector.tensor_add(s[:], ps[:, half:half + 256],
                                     sv_sb[:, 256 * b:256 * (b + 1)])
                out.append(s)
            return out

        def out_proj(it, b, pairs):
            for tcc in range(2):
                ps = psA.tile([128, 512], F32, tag="psA")
                for p in range(4):
                    mm(ps[:], pairs[p][:, 128 * tcc:128 * (tcc + 1)],
                       wo[p][:], start=(p == 0), stop=(p == 3))
                s = ypool.tile([128, 512], F16, tag="ys")
                nc.vector.tensor_copy(s[:], ps[:])
                nc.sync.dma_start(
                    y[2 * it + b, 128 * tcc:128 * (tcc + 1), :], s[:])

        def stage_a(it):
            qs = [proj_q(it, j) for j in range(4)]
            ks = proj_k(it)
            vs = proj_v(it)
            svs = proj_sv(it, vs)
            state[it] = (qs, ks, vs, svs)

        # ---------- software pipeline ----------
        # stage A of iteration 0 up front; then per iteration interleave the
        # next iteration's projections into the tanh-latency gaps of the
        # current iteration's attention so the PE never drains.
        stage_a(0)
        for it in range(NPAIR):
            qs, ks, vs, svs = state[it]
            nxt = it + 1 if it + 1 < NPAIR else None
            if nxt is not None and nxt + 1 < NPAIR:
                fetch_x(nxt + 1)

            # ---- bt b = 0 ----
            a0 = scores_j(qs, ks, 0, 0)
            a1 = scores_j(qs, ks, 0, 1)
            nqs = [None] * 4
            if nxt is not None:
                nqs[0] = proj_q(nxt, 0)
            a2 = scores_j(qs, ks, 0, 2)
            if nxt is not None:
                nqs[1] = proj_q(nxt, 1)
            a3 = scores_j(qs, ks, 0, 3)
            if nxt is not None:
                nqs[2] = proj_q(nxt, 2)
            pr = attnv_pair(vs, svs, 0, a0, a1)
            if nxt is not None:
                nqs[3] = proj_q(nxt, 3)
            pr += attnv_pair(vs, svs, 0, a2, a3)
            nks = proj_k(nxt) if nxt is not None else None
            out_proj(it, 0, pr)

            # ---- bt b = 1 ----
            b0 = scores_j(qs, ks, 1, 0)
            b1 = scores_j(qs, ks, 1, 1)
            nvs = proj_v(nxt) if nxt is not None else None
            b2 = scores_j(qs, ks, 1, 2)
            b3 = scores_j(qs, ks, 1, 3)
            nsvs = proj_sv(nxt, nvs) if nxt is not None else None
            pr = attnv_pair(vs, svs, 1, b0, b1)
            pr += attnv_pair(vs, svs, 1, b2, b3)
            out_proj(it, 1, pr)

            if nxt is not None:
                state[nxt] = (nqs, nks, nvs, nsvs)
            state[it] = None


def _get_runner():
    if "nc" not in _cached:
        _cached["nc"] = _build_nc()
    return _cached["nc"]


def _prep_inputs(x, sgr, Wq, Wk, Wv, Wo):
    f16 = np.float16
    x = np.asarray(x, dtype=np.float32)
    xb = x.reshape(BT, N, C)
    # head pair order [h0,h4 | h1,h5 | h2,h6 | h3,h7]
    perm = np.concatenate(
        [np.r_[64 * p:64 * (p + 1), 64 * (p + 4):64 * (p + 5)]
         for p in range(4)])
    Wqp = np.ascontiguousarray(np.asarray(Wq, dtype=np.float32)[:, perm]).astype(f16)
    Wop = np.ascontiguousarray(np.asarray(Wo, dtype=np.float32)[perm, :]).astype(f16)
    sgrT = np.ascontiguousarray(np.asarray(sgr, dtype=np.float32).T).astype(f16)
    Wk = np.ascontiguousarray(np.asarray(Wk, dtype=np.float32)).astype(f16)
    Wv = np.ascontiguousarray(np.asarray(Wv, dtype=np.float32)).astype(f16)

    in_maps = []
    for core in range(NCORES):
        xc = xb[PER_CORE * core: PER_CORE * (core + 1)]        # [16, 256, 512]
        xtc = xc.transpose(0, 2, 1)                            # [16, 512, 256]
        xarr = np.ascontiguousarray(
            xtc.reshape(NPAIR, 2, 4, 128, N)
               .transpose(0, 3, 2, 1, 4)
               .reshape(NPAIR, 128, 4, 512)).astype(f16)
        in_maps.append({
            "xT": xarr, "sgrT": sgrT, "Wqp": Wqp, "Wk": Wk,
            "Wv": Wv, "Wop": Wop,
        })
    return in_maps


def _run(x, sgr, Wq, Wk, Wv, Wo, trace=False, tmpdir=None):
    nc = _get_runner()
    in_maps = _prep_inputs(x, sgr, Wq, Wk, Wv, Wo)
    res = run_bass_kernel_spmd(nc, in_maps, list(range(NCORES)), trace=trace,
                               tmpdir=tmpdir)
    outs = [res.results[i]["y"] for i in range(NCORES)]
    full = np.concatenate(outs, axis=0).reshape(B, T, N, C).astype(np.float32)
    return full, res


def kernel(x, sgr, Wq, Wk, Wv, Wo):
    out, _ = _run(x, sgr, Wq, Wk, Wv, Wo, trace=False)
    return out
